# revision 1
# baseline (speedup 1.0000x reference)
"""DETR loss (cost matrix + Hungarian matching + losses) on 8 Trainium2 cores.

Sharding: data-parallel over batch. Each core handles 4 images as 2 pairs of 2
images packed into 128 SBUF partitions (2 images x 64 targets). The device
computes, per image, the [T=64, Q=300] matching-cost block (L1 cdist + class
cost + pairwise GIoU cost). The inherently serial Hungarian assignment runs on
host (exactly as in the reference, whose matcher is host-side numpy), and the
scalar loss is assembled on host from the matched pairs.
"""
import numpy as np

B, Q, T, C = 32, 300, 64, 2
N_CORES = 8
IMGS_PER_CORE = B // N_CORES          # 4
PAIRS_PER_CORE = IMGS_PER_CORE // 2   # 2
CLS_SCALE = 0.1
BBOX_SCALE = 5.0
GIOU_SCALE = 2.0

PIPE_DT = "bfloat16"   # dtype of the post-PSUM cost pipeline

# engine assignment knobs (tuned via CoreSim cost model)
R_ENGS = ["scalar", "scalar", "vector", "vector"]   # r1..r4
B_ENGS = ["scalar", "scalar", "scalar", "scalar"]   # b1..b4
TT_ENG = "gpsimd"    # LB / P2 / OUT adds
WE_ENG = "vector"

_CACHE = {}


def _split_wide_waits(nc, mybir, max_waits=1):
    """This walrus rejects instructions carrying >1 sem-wait; hoist extra
    waits onto NoOp carriers inserted just before (same engine, in-order)."""
    n_new = 0
    for bb in nc.main_func.blocks:
        insts = bb.instructions
        i = 0
        while i < len(insts):
            ins = insts[i]
            si = ins.sync_info
            if (
                si is not None
                and si.on_wait is not None
                and len(si.on_wait) > max_waits
            ):
                waits = list(si.on_wait)
                si.on_wait = waits[:max_waits]
                extra = waits[max_waits:]
                for j in range(0, len(extra), max_waits):
                    nd = mybir.InstNoOp(name=f"{ins.name}-xw{n_new}", ins=[], outs=[])
                    nd.engine = ins.engine
                    nd.sync_info = mybir.SyncInfo(
                        on_wait=extra[j : j + max_waits], on_update=[]
                    )
                    nc.register_instruction(nd, overwrite=True)
                    insts.insert(i, nd)
                    n_new += 1
                    i += 1
            i += 1
    return n_new


def _build_program():
    import concourse.bass as bass
    import concourse.mybir as mybir
    from concourse.tile import TileContext

    f32 = mybir.dt.float32
    DT = getattr(mybir.dt, PIPE_DT)
    op = mybir.AluOpType
    AF = mybir.ActivationFunctionType
    # qrows slots: px1, -px2, py1, -py2 | pcx, pcy, pw, ph | area1, f
    NQROW = 10
    # trows: ty1, nty2, tw, th, area2, ntx1, tx2, ntcx, ntcy, ntw, nth
    NTROW = 11
    QW3 = NQROW * Q

    bf16 = mybir.dt.bfloat16
    nc = bass.Bass()
    # per pair: 3 groups x 2 imgs x 4 quantity-slots of Q cols
    qrows = nc.declare_dram_parameter("qrows", [PAIRS_PER_CORE, 96, 4 * Q], bf16, isOutput=False)
    trows = nc.declare_dram_parameter("trows", [128, PAIRS_PER_CORE * NTROW], f32, isOutput=False)
    cost_o = nc.declare_dram_parameter("cost", [PAIRS_PER_CORE, 128, Q], DT, isOutput=True)

    with TileContext(nc) as tc:
        with (
            nc.allow_low_precision(reason="bf16 cost pipeline; assignment-tolerant"),
            tc.tile_pool(name="const", bufs=1) as cpool,
            tc.tile_pool(name="sb", bufs=2) as sb,
            tc.tile_pool(name="ps", bufs=4, space="PSUM") as ps,
        ):
            # indicator built on-chip at each legal matmul base (0/32/64):
            # row0 = [1]*64+[0]*64 (applied to A-B), row1 = all ones (applied to B)
            indt = cpool.tile([96, 128], bf16)
            for g in range(3):
                nc.vector.memset(indt[g * 32:g * 32 + 2, :], 1.0)
                nc.vector.memset(indt[g * 32:g * 32 + 1, 64:128], 0.0)
            # warm the ACT table set (Relu+Abs) while input DMAs are in flight
            warm = cpool.tile([2, 128], DT)
            nc.scalar.activation(warm[:], indt[0:2, :], AF.Relu)
            nc.scalar.activation(warm[:], indt[0:2, :], AF.Abs)

            # per-pair input DMA into partition groups at legal matmul bases
            qts = []
            for p in range(PAIRS_PER_CORE):
                qt = sb.tile([96, 4 * Q], bf16, tag=f"qt{p}")
                (nc.sync if p % 2 == 0 else nc.gpsimd).dma_start(out=qt[:], in_=qrows[p])
                qts.append(qt)
            trt = sb.tile([128, PAIRS_PER_CORE * NTROW], f32, tag="trt")
            nc.scalar.dma_start(out=trt[:], in_=trows[:])

            def mm_round(p, ks):
                Mr = ps.tile([128, 2 * 512], f32, tag="mega")
                Mrv = Mr[:].rearrange("p (s k) -> p s k", k=512)
                for i, k in enumerate(ks):
                    g, ck = (0, k) if k < 4 else ((1, k - 4) if k < 7 else (2, k - 7))
                    nc.tensor.matmul(Mrv[:, i, 0:Q], lhsT=indt[g * 32:g * 32 + 2, :],
                                     rhs=qts[p][g * 32:g * 32 + 2, ck * Q:(ck + 1) * Q],
                                     start=True, stop=True)
                return Mrv

            def fused(out_ap, psum_ap, bias_ap, kind, eng):
                if eng == "scalar":
                    nc.scalar.activation(out_ap, psum_ap,
                                         AF.Relu if kind == "relu" else AF.Abs,
                                         bias=bias_ap)
                else:
                    getattr(nc, eng).tensor_scalar(
                        out=out_ap, in0=psum_ap, scalar1=bias_ap, scalar2=0.0,
                        op0=op.add,
                        op1=op.max if kind == "relu" else op.abs_max)

            st = [dict() for _ in range(PAIRS_PER_CORE)]
            for p in range(PAIRS_PER_CORE):
                st[p]["Mx"] = mm_round(p, [0, 1])      # px1, -px2
                st[p]["My"] = mm_round(p, [2, 3])      # py1, -py2
            for p in range(PAIRS_PER_CORE):
                def sc(k, p=p):
                    return trt[:, p * NTROW + k:p * NTROW + k + 1]
                R13 = sb.tile([128, 2 * Q], DT, tag=f"R13_{p}")
                R24 = sb.tile([128, 2 * Q], DT, tag=f"R24_{p}")
                fused(R13[:, :Q], st[p]["Mx"][:, 0, 0:Q], sc(3), "relu", R_ENGS[0])
                fused(R24[:, :Q], st[p]["Mx"][:, 1, 0:Q], sc(4), "relu", R_ENGS[1])
                fused(R13[:, Q:], st[p]["My"][:, 0, 0:Q], sc(5), "relu", R_ENGS[2])
                fused(R24[:, Q:], st[p]["My"][:, 1, 0:Q], sc(6), "relu", R_ENGS[3])
                st[p]["R13"], st[p]["R24"] = R13, R24
                st[p]["Mc"] = mm_round(p, [4, 5])      # pcx, pcy
                st[p]["Mw"] = mm_round(p, [6, 7])      # pw, ph
            for p in range(PAIRS_PER_CORE):
                def sc(k, p=p):
                    return trt[:, p * NTROW + k:p * NTROW + k + 1]
                S = sb.tile([128, 2 * Q], DT, tag=f"S_{p}")
                (nc.vector if p % 2 == 0 else nc.gpsimd).tensor_tensor(
                    out=S[:], in0=st[p]["R13"][:], in1=st[p]["R24"][:], op=op.add)
                st[p]["S"] = S
                B12 = sb.tile([128, 2 * Q], DT, tag=f"B12_{p}")
                B34 = sb.tile([128, 2 * Q], DT, tag=f"B34_{p}")
                fused(B12[:, :Q], st[p]["Mc"][:, 0, 0:Q], sc(7), "abs", B_ENGS[0])
                fused(B12[:, Q:], st[p]["Mc"][:, 1, 0:Q], sc(8), "abs", B_ENGS[1])
                fused(B34[:, :Q], st[p]["Mw"][:, 0, 0:Q], sc(9), "abs", B_ENGS[2])
                fused(B34[:, Q:], st[p]["Mw"][:, 1, 0:Q], sc(10), "abs", B_ENGS[3])
                st[p]["B12"], st[p]["B34"] = B12, B34
            for p in range(PAIRS_PER_CORE):
                def sc(k, p=p):
                    return trt[:, p * NTROW + k:p * NTROW + k + 1]
                tt_eng = getattr(nc, TT_ENG)
                ve = nc.vector if p % 2 == 0 else nc.gpsimd   # alternate pairs across engines
                S = st[p]["S"]
                NW = sb.tile([128, 2 * Q], DT, tag=f"NW_{p}")
                nc.vector.tensor_scalar(out=NW[:, :Q], in0=S[:, :Q], scalar1=sc(0), scalar2=0.0,
                                        op0=op.subtract, op1=op.min)
                nc.vector.tensor_scalar(out=NW[:, Q:], in0=S[:, Q:], scalar1=sc(1), scalar2=0.0,
                                        op0=op.subtract, op1=op.min)
                WE = sb.tile([128, 2 * Q], DT, tag=f"WE_{p}")
                getattr(nc, WE_ENG).tensor_tensor(
                    out=WE[:].rearrange("p (a b) -> p a b", b=Q),
                    in0=S[:].rearrange("p (a b) -> p a b", b=Q),
                    in1=st[p]["Mw"][:, :, 0:Q], op=op.add)
                LB = sb.tile([128, 2 * Q], DT, tag=f"LB_{p}")
                tt_eng.tensor_tensor(out=LB[:], in0=st[p]["B12"][:], in1=st[p]["B34"][:], op=op.add)
                st[p]["Ml"] = mm_round(p, [8, 9])      # area1, f
                # area1|f to SBUF via ACT so tail ops can run off-PSUM on any engine
                FA = sb.tile([128, 2 * Q], DT, tag=f"FA_{p}")
                if p % 2 == 0:
                    nc.vector.tensor_copy(FA[:].rearrange("p (a b) -> p a b", b=Q),
                                          st[p]["Ml"][:, :, 0:Q])
                else:
                    nc.scalar.copy(out=FA[:].rearrange("p (a b) -> p a b", b=Q),
                                   in_=st[p]["Ml"][:, :, 0:Q])
                T1 = sb.tile([128, 2 * Q], DT, tag=f"T1_{p}")   # [inter | -union]
                ve.tensor_tensor(out=T1[:, :Q], in0=NW[:, :Q], in1=NW[:, Q:], op=op.mult)
                nc.vector.scalar_tensor_tensor(out=T1[:, Q:], in0=T1[:, :Q], scalar=sc(2),
                                               in1=FA[:, :Q], op0=op.subtract, op1=op.subtract)
                ENC = sb.tile([128, Q], DT, tag=f"ENC_{p}")
                ve.tensor_tensor(out=ENC[:], in0=WE[:, :Q], in1=WE[:, Q:], op=op.mult)
                IU = sb.tile([128, 2 * Q], DT, tag=f"IU_{p}")   # [-iou | -ue]
                RC = sb.tile([128, 2 * Q], DT, tag=f"RC_{p}")   # [1/-union | 1/enc]
                nc.vector.reciprocal(out=RC[:, :Q], in_=T1[:, Q:])
                nc.vector.reciprocal(out=RC[:, Q:], in_=ENC[:])
                nc.vector.tensor_tensor(out=IU[:, :Q], in0=T1[:, :Q], in1=RC[:, :Q], op=op.mult)
                nc.gpsimd.tensor_tensor(out=IU[:, Q:], in0=T1[:, Q:], in1=RC[:, Q:], op=op.mult)
                P1 = sb.tile([128, Q], DT, tag=f"P1_{p}")
                ve.tensor_tensor(out=P1[:], in0=IU[:, :Q], in1=IU[:, Q:], op=op.add)
                P2 = sb.tile([128, Q], DT, tag=f"P2_{p}")
                tt_eng.tensor_tensor(out=P2[:], in0=LB[:, :Q], in1=LB[:, Q:], op=op.add)
                P3 = sb.tile([128, Q], DT, tag=f"P3_{p}")
                ve.tensor_tensor(out=P3[:], in0=P2[:], in1=FA[:, Q:], op=op.add)
                OUT = sb.tile([128, Q], DT, tag=f"OUT_{p}")
                tt_eng.tensor_tensor(out=OUT[:], in0=P3[:], in1=P1[:], op=op.add)
                (nc.sync if p % 2 == 0 else nc.scalar).dma_start(out=cost_o[p], in_=OUT[:])

    _split_wide_waits(nc, mybir)
    return nc


def _lsa(cost):
    # Hungarian (shortest augmenting path), identical algorithm to reference.
    cost = np.asarray(cost, dtype=np.float64)
    n, m = cost.shape
    u = np.zeros(n + 1)
    v = np.zeros(m + 1)
    p = np.zeros(m + 1, dtype=np.int64)
    way = np.zeros(m + 1, dtype=np.int64)
    for i in range(1, n + 1):
        p[0] = i
        j0 = 0
        minv = np.full(m + 1, np.inf)
        used = np.zeros(m + 1, dtype=bool)
        while True:
            used[j0] = True
            i0 = p[j0]
            cur = cost[i0 - 1, :] - u[i0] - v[1:]
            free = ~used[1:]
            upd = free & (cur < minv[1:])
            minv[1:][upd] = cur[upd]
            way[1:][upd] = j0
            cand = np.where(free, minv[1:], np.inf)
            j1 = int(np.argmin(cand)) + 1
            delta = cand[j1 - 1]
            u[p[used]] += delta
            v[used] -= delta
            minv[~used] -= delta
            j0 = j1
            if p[j0] == 0:
                break
        while j0:
            j1 = way[j0]
            p[j0] = p[j1]
            j0 = j1
    ans = np.zeros(n, dtype=np.int64)
    for j in range(1, m + 1):
        if p[j] > 0:
            ans[p[j] - 1] = j - 1
    return ans


def _host_prep(logits, pred_bbox, target_bbox):
    import ml_dtypes
    logits = np.ascontiguousarray(logits, np.float32)
    pb = np.ascontiguousarray(pred_bbox, np.float32)
    tb = np.ascontiguousarray(target_bbox, np.float32)

    pcx, pcy, pw, ph = pb[..., 0], pb[..., 1], pb[..., 2], pb[..., 3]
    px1, py1 = pcx - 0.5 * pw, pcy - 0.5 * ph
    px2, py2 = pcx + 0.5 * pw, pcy + 0.5 * ph
    area1 = pw * ph
    dl = (logits[..., 1] - logits[..., 0]).astype(np.float64)
    f = (1.0 / (1.0 + np.exp(-dl))).astype(np.float32)   # 1 - p0 = sigmoid(l1-l0)
    # [B, 10, Q], quantity-major
    qr_all = np.stack([px1, -px2, py1, -py2, pcx, pcy, pw, ph, area1, f], axis=1)

    tcx, tcy, tw, th = tb[..., 0], tb[..., 1], tb[..., 2], tb[..., 3]
    tx1, ty1 = tcx - 0.5 * tw, tcy - 0.5 * th
    tx2, ty2 = tcx + 0.5 * tw, tcy + 0.5 * th
    area2 = tw * th
    # [B, T, 11]
    tr_all = np.stack([tw, th, area2, -tx1, tx2, -ty1, ty2, -tcx, -tcy, -tw, -th],
                      axis=-1)

    in_maps = []
    for c in range(N_CORES):
        i0 = c * IMGS_PER_CORE
        # qrows: [pair, group(3) x img(2), 4*Q] bf16, groups of quantities
        qc4 = qr_all[i0:i0 + IMGS_PER_CORE].reshape(PAIRS_PER_CORE, 2, 10, Q)
        # pre-round to bf16 so the A-B row is an exact difference of bf16 values
        qc4 = qc4.astype(ml_dtypes.bfloat16).astype(np.float32)
        qc = np.zeros((PAIRS_PER_CORE, 96, 4 * Q), np.float32)
        for g, ks in enumerate(([0, 1, 2, 3], [4, 5, 6], [7, 8, 9])):
            for j, k in enumerate(ks):
                # row0 = imgA - imgB (selected on partitions 0-63), row1 = imgB
                qc[:, g * 32 + 0, j * Q:(j + 1) * Q] = qc4[:, 0, k, :] - qc4[:, 1, k, :]
                qc[:, g * 32 + 1, j * Q:(j + 1) * Q] = qc4[:, 1, k, :]
        # trows: [128 partitions, pair*11]
        tc_ = tr_all[i0:i0 + IMGS_PER_CORE].reshape(PAIRS_PER_CORE, 128, 11)
        tc_ = tc_.transpose(1, 0, 2).reshape(128, PAIRS_PER_CORE * 11)
        in_maps.append({
            "qrows": np.ascontiguousarray(qc).astype(ml_dtypes.bfloat16),
            "trows": np.ascontiguousarray(tc_),
        })
    return in_maps


def _finalize(logits, pred_bbox, target_bbox, target_labels, src):
    labels = np.asarray(target_labels).astype(np.int64)
    lg = np.asarray(logits, np.float64)
    pb = np.asarray(pred_bbox, np.float64)
    tb = np.asarray(target_bbox, np.float64)
    bidx = np.arange(B)[:, None]

    # CE pieces (exact, host): nlpk = -logp_k
    dl = lg[..., 1] - lg[..., 0]
    nlp1 = np.logaddexp(0.0, -dl)       # -logp1 = softplus(l0-l1)
    nlp0 = np.logaddexp(0.0, dl)        # -logp0 = softplus(l1-l0)
    g = nlp0 - CLS_SCALE * nlp1         # matched-query correction (labels are 0)
    A = nlp1.sum()
    w = np.ones(C); w[-1] = CLS_SCALE
    wt_sum = CLS_SCALE * (B * Q) + np.sum(w[labels] - CLS_SCALE)
    ce = (CLS_SCALE * A + g[bidx, src].sum()) / wt_sum

    mp = pb[bidx, src].reshape(-1, 4)
    mt = tb.reshape(-1, 4)
    nb = B * T
    l1 = np.abs(mp - mt).sum() / nb

    def corners(x):
        cx, cy, ww, hh = x[:, 0], x[:, 1], x[:, 2], x[:, 3]
        return np.stack([cx - .5 * ww, cy - .5 * hh, cx + .5 * ww, cy + .5 * hh], -1)

    c1, c2 = corners(mp), corners(mt)
    a1 = (c1[:, 2] - c1[:, 0]) * (c1[:, 3] - c1[:, 1])
    a2 = (c2[:, 2] - c2[:, 0]) * (c2[:, 3] - c2[:, 1])
    lt = np.maximum(c1[:, :2], c2[:, :2]); rb = np.minimum(c1[:, 2:], c2[:, 2:])
    wh = np.clip(rb - lt, 0, None); inter = wh[:, 0] * wh[:, 1]
    union = a1 + a2 - inter
    iou = inter / union
    lte = np.minimum(c1[:, :2], c2[:, :2]); rbe = np.maximum(c1[:, 2:], c2[:, 2:])
    whe = np.clip(rbe - lte, 0, None); encl = whe[:, 0] * whe[:, 1]
    giou = iou - (encl - union) / encl
    lgi = (1.0 - giou).sum() / nb
    return ce + BBOX_SCALE * l1 + GIOU_SCALE * lgi


def kernel(logits, pred_bbox, target_bbox, target_labels):
    import os
    os.environ["BASS_NEVER_TRACE"] = "1"   # no NTFF hook in this container
    from concourse.bass_utils import run_bass_kernel_spmd

    if "nc" not in _CACHE:
        _CACHE["nc"] = _build_program()
    nc = _CACHE["nc"]

    in_maps = _host_prep(logits, pred_bbox, target_bbox)
    res = run_bass_kernel_spmd(nc, in_maps, core_ids=list(range(N_CORES)))
    _CACHE["last_res"] = res

    cost_T = np.zeros((B, T, Q), np.float32)   # [img, target, query]
    for c in range(N_CORES):
        cb = np.asarray(res.results[c]["cost"]).astype(np.float32).reshape(PAIRS_PER_CORE, 2, 64, Q)
        i0 = c * IMGS_PER_CORE
        for p in range(PAIRS_PER_CORE):
            cost_T[i0 + 2 * p] = cb[p, 0]
            cost_T[i0 + 2 * p + 1] = cb[p, 1]

    src = np.zeros((B, T), np.int64)
    for i in range(B):
        src[i] = _lsa(cost_T[i])

    total = _finalize(logits, pred_bbox, target_bbox, target_labels, src)
    return np.float32(total)



# revision 11
# speedup vs baseline: 1.2603x; 1.2603x over previous
"""DETR loss (cost matrix + Hungarian matching + losses) on 8 Trainium2 cores.

Sharding: data-parallel over batch. Each core handles 4 images as 2 pairs of 2
images packed into 128 SBUF partitions (2 images x 64 targets). Per pair the
device computes the [128, Q=300] matching-cost block:

  cost[t,q] = L1(bbox) - iou - union/enclose     (+ f[q] added on host;
                                                  constant offsets cancel)

The pairwise terms are built from PE broadcasts: for each per-query quantity a
K=2/3 matmul broadcasts it across the 128 target partitions, with per-target
biases folded into a third lhsT row where the downstream op could not apply
them (X2/Y2 for the relu-sum, CX/CY/DW/DH for the L1 abs terms). Post-PSUM
work is split across Pool (relu/abs folds, unions), ACT (abs duos), and DVE
(clips, products, a fused tensor-tensor divide for iou|union/enclose).

The inherently serial Hungarian assignment runs on host (as in the reference,
whose matcher is host-side numpy), and the scalar loss is assembled on host
from the matched pairs in f64.
"""
import numpy as np

B, Q, T, C = 32, 300, 64, 2
N_CORES = 8
IMGS_PER_CORE = B // N_CORES          # 4
PAIRS_PER_CORE = IMGS_PER_CORE // 2   # 2
CLS_SCALE = 0.1
BBOX_SCALE = 5.0
GIOU_SCALE = 2.0

# 3 matmul groups at bases 0/32/64; rows base+0..2 = pair0 [A-B, B, ones],
# rows base+3..5 = pair1. Slot columns are shared across pairs (the lhsT
# blocks zero out the other pair's rows). Column layouts (bf16 cols):
#  g0: slots -px2@0, -py2@300, pcx@600;
#      lhsT blocks: X2p0@900, X2p1@1028, Y2p0@1156, Y2p1@1284,
#                   CXp0@1412, CXp1@1540                      -> 1668 cols
#  g1: slots pw@0, ph@300;
#      lhsT: purep0@600, purep1@728, DWp0@856, DWp1@984,
#            DHp0@1112, DHp1@1240                             -> 1368 cols
#  g2: slots px1@0, py1@300, area1@600, pcy@900;
#      lhsT: purep0@1200, purep1@1328, CYp0@1456, CYp1@1584   -> 1712 cols
QCOLS = 1712
QROWS = 70

# engine knobs for the elementwise stages (tuned on CoreSim)
ENG = {
    "o1": "gpsimd", "o2": "gpsimd", "o3": "gpsimd", "o4": "gpsimd",
    "o5": "gpsimd", "o6": "gpsimd", "o7": "gpsimd",
    "nx": "vector", "ny": "vector", "inter": "vector", "enc": "vector",
    "lh": "vector", "vdiv": "vector", "p": "vector", "out": "vector",
}

_CACHE = {}


def _split_wide_waits(nc, mybir, max_waits=1):
    """Walrus rejects instructions carrying >1 sem-wait; hoist extra waits
    onto NoOp carriers inserted just before (same engine, in-order)."""
    n_new = 0
    for bb in nc.main_func.blocks:
        insts = bb.instructions
        i = 0
        while i < len(insts):
            ins = insts[i]
            si = ins.sync_info
            if (
                si is not None
                and si.on_wait is not None
                and len(si.on_wait) > max_waits
            ):
                waits = list(si.on_wait)
                si.on_wait = waits[:max_waits]
                extra = waits[max_waits:]
                for j in range(0, len(extra), max_waits):
                    nd = mybir.InstNoOp(name=f"{ins.name}-xw{n_new}", ins=[], outs=[])
                    nd.engine = ins.engine
                    nd.sync_info = mybir.SyncInfo(
                        on_wait=extra[j : j + max_waits], on_update=[]
                    )
                    nc.register_instruction(nd, overwrite=True)
                    insts.insert(i, nd)
                    n_new += 1
                    i += 1
            i += 1
    return n_new


def _build_program():
    import concourse.bass as bass
    import concourse.mybir as mybir
    from concourse.tile import TileContext

    f32 = mybir.dt.float32
    bf16 = mybir.dt.bfloat16
    op = mybir.AluOpType
    AF = mybir.ActivationFunctionType

    nc = bass.Bass()
    qin = nc.declare_dram_parameter("qin", [QROWS, QCOLS], bf16, isOutput=False)
    scal = nc.declare_dram_parameter("scal", [128, 10], f32, isOutput=False)
    cost_o = nc.declare_dram_parameter("cost", [128, 2 * Q], bf16, isOutput=True)

    def eng(key):
        return getattr(nc, ENG[key])

    with TileContext(nc) as tc:
        with (
            nc.allow_low_precision(reason="bf16 cost pipeline; assignment-tolerant"),
            tc.tile_pool(name="sb", bufs=1) as sb,
            tc.tile_pool(name="ps", bufs=3, space="PSUM") as ps,
            tc.tile_pool(name="pa", bufs=2, space="PSUM") as pa,
        ):
            qt = sb.tile([QROWS, QCOLS], bf16, tag="qt")
            # input DMA in 2 parallel column chunks (SP + ACT hwdge queues)
            c1 = 856
            nc.sync.dma_start(out=qt[:, 0:c1], in_=qin[:, 0:c1])
            nc.scalar.dma_start(out=qt[:, c1:QCOLS], in_=qin[:, c1:QCOLS])
            sct = sb.tile([128, 10], f32, tag="sct")
            nc.sync.dma_start(out=sct[:], in_=scal[:])

            # warm the ACT table (Abs lives in every set) during DMA flight
            warm = sb.tile([2, 128], bf16, tag="warm")
            nc.scalar.activation(warm[:], qt[0:2, 0:128], AF.Abs)

            # per-pair scalar APs: [tx1, ty1, area2, tw, th] at cols 5p..5p+5
            def sc(p, k):
                return sct[:, 5 * p + k:5 * p + k + 1]

            st = [dict() for _ in range(PAIRS_PER_CORE)]

            # ---- matmuls -------------------------------------------------
            # pair0 windows: K=2 (pure) / K=3 (bias); pair1: K=5 / K=6 with
            # zeroed pair0 rows in the lhsT block.
            def mm(out_ap, gbase, lcol, scol, p, bias):
                k = (3 if bias else 2) + 3 * p
                nc.tensor.matmul(out_ap, lhsT=qt[gbase:gbase + k, lcol:lcol + 128],
                                 rhs=qt[gbase:gbase + k, scol:scol + 300],
                                 start=True, stop=True)

            def lcolp(base, p):
                return base + 128 * p

            # A1 first (dedicated banks; consumed late by o7)
            for p in range(PAIRS_PER_CORE):
                ra = pa.tile([128, 512], f32, tag="ra")
                mm(ra[:, 0:Q], 64, lcolp(1200, p), 600, p, False)   # area1
                st[p]["A1"] = ra

            # R1: [X1, X2]  X1 = px1 (pure), X2 = tx2 - px2 (bias)
            # R2: [Y1, Y2]
            for p in range(PAIRS_PER_CORE):
                r = ps.tile([128, 1024], f32, tag="mm2")
                st[p]["R1"] = r
                rv = r[:].rearrange("q (s k) -> q s k", k=512)
                mm(rv[:, 0, 0:Q], 64, lcolp(1200, p), 0, p, False)  # X1 = px1
                mm(rv[:, 1, 0:Q], 0, lcolp(900, p), 0, p, True)     # X2 = tx2-px2
                st[p]["R1v"] = rv
                r2 = ps.tile([128, 1024], f32, tag="mm2")
                st[p]["R2"] = r2
                rv2 = r2[:].rearrange("q (s k) -> q s k", k=512)
                mm(rv2[:, 0, 0:Q], 64, lcolp(1200, p), 300, p, False)  # Y1 = py1
                mm(rv2[:, 1, 0:Q], 0, lcolp(1156, p), 300, p, True)    # Y2
                st[p]["R2v"] = rv2

            # ---- relu folds (consume R1/R2) ------------------------------
            for p in range(PAIRS_PER_CORE):
                RX1 = sb.tile([128, Q], bf16, tag=f"RX1_{p}")
                eng("o1").tensor_scalar(out=RX1[:], in0=st[p]["R1v"][:, 0, 0:Q],
                                        scalar1=sc(p, 0), scalar2=0.0,
                                        op0=op.subtract, op1=op.max)
                Sx = sb.tile([128, Q], bf16, tag=f"Sx_{p}")
                eng("o2").scalar_tensor_tensor(out=Sx[:], in0=st[p]["R1v"][:, 1, 0:Q],
                                               scalar=0.0, in1=RX1[:],
                                               op0=op.max, op1=op.add)
                RY1 = sb.tile([128, Q], bf16, tag=f"RY1_{p}")
                eng("o3").tensor_scalar(out=RY1[:], in0=st[p]["R2v"][:, 0, 0:Q],
                                        scalar1=sc(p, 1), scalar2=0.0,
                                        op0=op.subtract, op1=op.max)
                Sy = sb.tile([128, Q], bf16, tag=f"Sy_{p}")
                eng("o4").scalar_tensor_tensor(out=Sy[:], in0=st[p]["R2v"][:, 1, 0:Q],
                                               scalar=0.0, in1=RY1[:],
                                               op0=op.max, op1=op.add)
                st[p]["Sx"], st[p]["Sy"] = Sx, Sy

            # ---- remaining matmul rounds + their consumers ---------------
            for p in range(PAIRS_PER_CORE):
                # R3: [PW, PH] (pure)
                r3 = ps.tile([128, 1024], f32, tag="mm2")
                r3v = r3[:].rearrange("q (s k) -> q s k", k=512)
                mm(r3v[:, 0, 0:Q], 32, lcolp(600, p), 0, p, False)   # PW = pw
                mm(r3v[:, 1, 0:Q], 32, lcolp(600, p), 300, p, False)  # PH = ph

                IUE = sb.tile([128, 3 * Q], bf16, tag=f"IUE_{p}")
                st[p]["IUE"] = IUE
                Wx = sb.tile([128, Q], bf16, tag=f"Wx_{p}")
                eng("o5").tensor_tensor(out=Wx[:], in0=st[p]["Sx"][:],
                                        in1=r3v[:, 0, 0:Q], op=op.add)
                Wy = sb.tile([128, Q], bf16, tag=f"Wy_{p}")
                eng("o6").tensor_tensor(out=Wy[:], in0=st[p]["Sy"][:],
                                        in1=r3v[:, 1, 0:Q], op=op.add)

                NX = sb.tile([128, Q], bf16, tag=f"NX_{p}")
                eng("nx").tensor_scalar(out=NX[:], in0=st[p]["Sx"][:],
                                        scalar1=sc(p, 3), scalar2=0.0,
                                        op0=op.subtract, op1=op.min)
                NY = sb.tile([128, Q], bf16, tag=f"NY_{p}")
                eng("ny").tensor_scalar(out=NY[:], in0=st[p]["Sy"][:],
                                        scalar1=sc(p, 4), scalar2=0.0,
                                        op0=op.subtract, op1=op.min)
                eng("inter").tensor_tensor(out=IUE[:, 0:Q], in0=NX[:], in1=NY[:],
                                           op=op.mult)
                eng("enc").tensor_tensor(out=IUE[:, 2 * Q:3 * Q], in0=Wx[:],
                                         in1=Wy[:], op=op.mult)
                # union = (A1 + area2) - inter
                eng("o7").scalar_tensor_tensor(out=IUE[:, Q:2 * Q],
                                               in0=st[p]["A1"][:, 0:Q],
                                               scalar=sc(p, 2), in1=IUE[:, 0:Q],
                                               op0=op.add, op1=op.subtract)

            for p in range(PAIRS_PER_CORE):
                # R4: [CX, CY] (bias), R5: [DW, DH] (bias)
                r4 = ps.tile([128, 1024], f32, tag="mm2")
                r4v = r4[:].rearrange("q (s k) -> q s k", k=512)
                mm(r4v[:, 0, 0:Q], 0, lcolp(1412, p), 600, p, True)   # CX
                mm(r4v[:, 1, 0:Q], 64, lcolp(1456, p), 900, p, True)  # CY
                r5 = ps.tile([128, 1024], f32, tag="mm2")
                r5v = r5[:].rearrange("q (s k) -> q s k", k=512)
                mm(r5v[:, 0, 0:Q], 32, lcolp(856, p), 0, p, True)     # DW
                mm(r5v[:, 1, 0:Q], 32, lcolp(1112, p), 300, p, True)  # DH

                AB12 = sb.tile([128, 2 * Q], bf16, tag=f"AB12_{p}")
                nc.scalar.activation(AB12[:].rearrange("q (s k) -> q s k", k=Q),
                                     r4v[:, :, 0:Q], AF.Abs)
                AB34 = sb.tile([128, 2 * Q], bf16, tag=f"AB34_{p}")
                nc.scalar.activation(AB34[:].rearrange("q (s k) -> q s k", k=Q),
                                     r5v[:, :, 0:Q], AF.Abs)
                Lh = sb.tile([128, 2 * Q], bf16, tag=f"Lh_{p}")
                eng("lh").tensor_tensor(out=Lh[:], in0=AB12[:], in1=AB34[:],
                                        op=op.add)
                st[p]["Lh"] = Lh

            OUTT = sb.tile([128, 2 * Q], bf16, tag="OUTT")
            for p in range(PAIRS_PER_CORE):
                IUE = st[p]["IUE"]
                V = sb.tile([128, 2 * Q], bf16, tag=f"V_{p}")
                eng("vdiv").tensor_tensor(out=V[:], in0=IUE[:, 0:2 * Q],
                                          in1=IUE[:, Q:3 * Q], op=op.divide)
                P = sb.tile([128, 2 * Q], bf16, tag=f"P_{p}")
                eng("p").tensor_tensor(out=P[:], in0=st[p]["Lh"][:], in1=V[:],
                                       op=op.subtract)
                eng("out").tensor_tensor(out=OUTT[:, Q * p:Q * (p + 1)],
                                         in0=P[:, 0:Q], in1=P[:, Q:2 * Q],
                                         op=op.add)
            nc.sync.dma_start(out=cost_o[:, 0:Q], in_=OUTT[:, 0:Q])
            nc.scalar.dma_start(out=cost_o[:, Q:2 * Q], in_=OUTT[:, Q:2 * Q])

    _split_wide_waits(nc, mybir)
    return nc


def _lsa(cost):
    # Hungarian (shortest augmenting path), identical algorithm to reference.
    cost = np.asarray(cost, dtype=np.float64)
    n, m = cost.shape
    u = np.zeros(n + 1)
    v = np.zeros(m + 1)
    p = np.zeros(m + 1, dtype=np.int64)
    way = np.zeros(m + 1, dtype=np.int64)
    for i in range(1, n + 1):
        p[0] = i
        j0 = 0
        minv = np.full(m + 1, np.inf)
        used = np.zeros(m + 1, dtype=bool)
        while True:
            used[j0] = True
            i0 = p[j0]
            cur = cost[i0 - 1, :] - u[i0] - v[1:]
            free = ~used[1:]
            upd = free & (cur < minv[1:])
            minv[1:][upd] = cur[upd]
            way[1:][upd] = j0
            cand = np.where(free, minv[1:], np.inf)
            j1 = int(np.argmin(cand)) + 1
            delta = cand[j1 - 1]
            u[p[used]] += delta
            v[used] -= delta
            minv[~used] -= delta
            j0 = j1
            if p[j0] == 0:
                break
        while j0:
            j1 = way[j0]
            p[j0] = p[j1]
            j0 = j1
    ans = np.zeros(n, dtype=np.int64)
    for j in range(1, m + 1):
        if p[j] > 0:
            ans[p[j] - 1] = j - 1
    return ans


def _host_prep(logits, pred_bbox, target_bbox):
    import ml_dtypes
    pb = np.ascontiguousarray(pred_bbox, np.float32)
    tb = np.ascontiguousarray(target_bbox, np.float32)

    def rb(x):  # round to bf16, keep f32
        return x.astype(ml_dtypes.bfloat16).astype(np.float32)

    pcx, pcy, pw, ph = rb(pb[..., 0]), rb(pb[..., 1]), rb(pb[..., 2]), rb(pb[..., 3])
    px1, py1 = rb(pcx - 0.5 * pw), rb(pcy - 0.5 * ph)
    px2, py2 = rb(pcx + 0.5 * pw), rb(pcy + 0.5 * ph)
    area1 = rb(pw * ph)
    # slot data per group [B, nslots, Q]
    g0_slots = np.stack([-px2, -py2, pcx], axis=1)
    g1_slots = np.stack([pw, ph], axis=1)
    g2_slots = np.stack([px1, py1, area1, pcy], axis=1)

    tcx, tcy, tw, th = tb[..., 0], tb[..., 1], tb[..., 2], tb[..., 3]
    tx1, ty1 = tcx - 0.5 * tw, tcy - 0.5 * th
    tx2, ty2 = tcx + 0.5 * tw, tcy + 0.5 * th
    area2 = tw * th

    ind = np.concatenate([np.ones(64, np.float32), np.zeros(64, np.float32)])
    ones128 = np.ones(128, np.float32)

    in_maps = []
    for c in range(N_CORES):
        qin = np.zeros((QROWS, QCOLS), np.float32)
        scal = np.zeros((128, 10), np.float32)
        for p in range(PAIRS_PER_CORE):
            ia, ib = c * IMGS_PER_CORE + 2 * p, c * IMGS_PER_CORE + 2 * p + 1
            # per-target vectors on 128 partitions: imgA targets 0:64, imgB 64:128
            def tvec(arr):
                return np.concatenate([arr[ia], arr[ib]]).astype(np.float32)

            # rows base+3p..base+3p+2 = [A-B, B, ones] of pair p
            for gbase, slots in ((0, g0_slots), (32, g1_slots), (64, g2_slots)):
                n = slots.shape[1] * Q
                qin[gbase + 3 * p + 0, 0:n] = (slots[ia] - slots[ib]).reshape(-1)
                qin[gbase + 3 * p + 1, 0:n] = slots[ib].reshape(-1)
                qin[gbase + 3 * p + 2, 0:n] = 1.0
            # lhsT blocks (128 cols each) at rows base+3p..base+3p+2:
            #  (gbase, col, bias-or-None)
            for gbase, col, bias in (
                (0, 900, tvec(tx2)), (0, 1156, tvec(ty2)), (0, 1412, -tvec(tcx)),
                (32, 600, None), (32, 856, -tvec(tw)), (32, 1112, -tvec(th)),
                (64, 1200, None), (64, 1456, -tvec(tcy)),
            ):
                cc = col + 128 * p
                qin[gbase + 3 * p + 0, cc:cc + 128] = ind
                qin[gbase + 3 * p + 1, cc:cc + 128] = ones128
                if bias is not None:
                    qin[gbase + 3 * p + 2, cc:cc + 128] = bias
            # scalars: [tx1, ty1, area2, tw, th] at cols 5p..5p+5
            scal[:, 5 * p + 0] = tvec(tx1)
            scal[:, 5 * p + 1] = tvec(ty1)
            scal[:, 5 * p + 2] = tvec(area2)
            scal[:, 5 * p + 3] = tvec(tw)
            scal[:, 5 * p + 4] = tvec(th)
        in_maps.append({
            "qin": qin.astype(ml_dtypes.bfloat16),
            "scal": np.ascontiguousarray(scal),
        })
    return in_maps


def _finalize(logits, pred_bbox, target_bbox, target_labels, src):
    labels = np.asarray(target_labels).astype(np.int64)
    lg = np.asarray(logits, np.float64)
    pb = np.asarray(pred_bbox, np.float64)
    tb = np.asarray(target_bbox, np.float64)
    bidx = np.arange(B)[:, None]

    # CE pieces (exact, host): nlpk = -logp_k
    dl = lg[..., 1] - lg[..., 0]
    nlp1 = np.logaddexp(0.0, -dl)       # -logp1 = softplus(l0-l1)
    nlp0 = np.logaddexp(0.0, dl)        # -logp0 = softplus(l1-l0)
    g = nlp0 - CLS_SCALE * nlp1         # matched-query correction (labels are 0)
    A = nlp1.sum()
    w = np.ones(C); w[-1] = CLS_SCALE
    wt_sum = CLS_SCALE * (B * Q) + np.sum(w[labels] - CLS_SCALE)
    ce = (CLS_SCALE * A + g[bidx, src].sum()) / wt_sum

    mp = pb[bidx, src].reshape(-1, 4)
    mt = tb.reshape(-1, 4)
    nb = B * T
    l1 = np.abs(mp - mt).sum() / nb

    def corners(x):
        cx, cy, ww, hh = x[:, 0], x[:, 1], x[:, 2], x[:, 3]
        return np.stack([cx - .5 * ww, cy - .5 * hh, cx + .5 * ww, cy + .5 * hh], -1)

    c1, c2 = corners(mp), corners(mt)
    a1 = (c1[:, 2] - c1[:, 0]) * (c1[:, 3] - c1[:, 1])
    a2 = (c2[:, 2] - c2[:, 0]) * (c2[:, 3] - c2[:, 1])
    lt = np.maximum(c1[:, :2], c2[:, :2]); rb = np.minimum(c1[:, 2:], c2[:, 2:])
    wh = np.clip(rb - lt, 0, None); inter = wh[:, 0] * wh[:, 1]
    union = a1 + a2 - inter
    iou = inter / union
    lte = np.minimum(c1[:, :2], c2[:, :2]); rbe = np.maximum(c1[:, 2:], c2[:, 2:])
    whe = np.clip(rbe - lte, 0, None); encl = whe[:, 0] * whe[:, 1]
    giou = iou - (encl - union) / encl
    lgi = (1.0 - giou).sum() / nb
    return ce + BBOX_SCALE * l1 + GIOU_SCALE * lgi


def kernel(logits, pred_bbox, target_bbox, target_labels):
    import os
    os.environ["BASS_NEVER_TRACE"] = "1"   # no NTFF hook in this container
    from concourse.bass_utils import run_bass_kernel_spmd

    if "nc" not in _CACHE:
        _CACHE["nc"] = _build_program()
    nc = _CACHE["nc"]

    in_maps = _host_prep(logits, pred_bbox, target_bbox)
    res = run_bass_kernel_spmd(nc, in_maps, core_ids=list(range(N_CORES)))
    _CACHE["last_res"] = res

    # class cost: per-query additive f = p1 = sigmoid(l1 - l0); constants cancel
    lg = np.asarray(logits, np.float64)
    f = 1.0 / (1.0 + np.exp(-(lg[..., 1] - lg[..., 0])))   # [B, Q]

    src = np.zeros((B, T), np.int64)
    for c in range(N_CORES):
        cb = np.asarray(res.results[c]["cost"]).astype(np.float32)  # [128, 600]
        for p in range(PAIRS_PER_CORE):
            for a in range(2):
                i = c * IMGS_PER_CORE + 2 * p + a
                block = cb[64 * a:64 * (a + 1), Q * p:Q * (p + 1)] + f[i][None, :]
                src[i] = _lsa(block)

    total = _finalize(logits, pred_bbox, target_bbox, target_labels, src)
    return np.float32(total)
